# revision 19
# baseline (speedup 1.0000x reference)
"""Block-sparse local+strided attention (LocalStridedBlockSparseAttn) on 8 trn2 cores.

Problem: q,k,v [4096, 16, 64] f32, single prefill sequence. Per-head block mask
(64x64 token blocks): j <= i and (i - j < 8  or  (j + h + 1) % 8 == 0).

Sharding: core c owns heads {c, c+8} - both have the same strided residue
s = (7 - c) % 8, so one SPMD program serves all 8 cores with per-core data.

v4 (instruction-minimal dataflow; v3 was PE-bound on per-instruction fixed
costs and startup DMA serialization):
  - local part per (chunk, head): one [128, w] matmul per k-block PAIR over
    its contiguous valid q-window (w up to 512), masked post-exp with small
    constant masks, plus one 64x64 tail piece; small pieces packed into
    shared PSUM banks so ONE activation serves several matmuls.
  - strided validity boundary folded into the CONTRACTION: ks carries 7
    indicator partitions and the strided q copy carries -1e9 rows on the
    boundary-chunk prefix columns (zero per-piece instructions).
  - output stays TRANSPOSED with the rowsums row: the [65, 512] PSUM tile
    [O^T; rowsums] is DMA'd straight to DRAM; the host does the divide and
    the final transpose. No PE transposes, no reciprocal/normalize/copy
    instructions on device.
  - big inputs split in halves with chunk-0-critical slices DMA'd first so
    compute starts before the tail of the input load.
All matmul operands bf16; exp outputs bf16 (PSUM accumulates fp32).
"""

import numpy as np

N_HEADS = 16
HEAD = 64
SEQ = 4096
BS = 64
NB = 64          # 64 token-blocks
LOCAL = 8
VERT = 8
SM_SCALE = 1.0 / 8.0
NCORES = 8
CHUNK = 512      # q tokens per chunk (8 blocks)
NCH = SEQ // CHUNK
NSB = 8          # packed strided block slots (7 real, slot 7 zero pad)
KSP = 64 + 7     # ks/qs partitions: 64 head dims + 7 boundary indicator rows
HSEQ = SEQ // 2

_cache = {}


def _legalize_waits(nc, max_waits=1):
    """This walrus build rejects instructions carrying more than one sync-wait
    condition ("Too many sync wait commands"); hoist extras into same-engine
    NoOps placed immediately before the instruction."""
    import concourse.mybir as mybir

    nid = 0
    for bb in nc.main_func.blocks:
        new = []
        for ins in bb.instructions:
            si = ins.sync_info
            if si is not None and si.on_wait and len(si.on_wait) > max_waits:
                waits = list(si.on_wait)
                while len(waits) > max_waits:
                    chunk, waits = waits[:max_waits], waits[max_waits:]
                    nid += 1
                    nop = mybir.InstNoOp(name=f"{ins.name}-wsplit{nid}")
                    nop.engine = ins.engine
                    nop.sync_info = mybir.SyncInfo(on_wait=chunk, on_update=[])
                    new.append(nop)
                ins.sync_info = mybir.SyncInfo(on_wait=waits,
                                               on_update=list(si.on_update))
            new.append(ins)
        bb.instructions[:] = new
    return nc


def _build_program(chunks=None, heads=(0, 1)):
    from contextlib import ExitStack

    import concourse.bass as bass
    import concourse.mybir as mybir
    from concourse import tile

    f32 = mybir.dt.float32
    bf16 = mybir.dt.bfloat16
    Exp = mybir.ActivationFunctionType.Exp

    nc = bass.Bass()
    qT_d = nc.dram_tensor("qT", [128, SEQ], bf16, kind="ExternalInput")
    kT_d = nc.dram_tensor("kT", [128, SEQ], bf16, kind="ExternalInput")
    qs_d = [nc.dram_tensor(f"qs{h}", [KSP, SEQ], bf16, kind="ExternalInput")
            for h in range(2)]
    ks_d = [nc.dram_tensor(f"ks{h}", [KSP, NSB * BS], bf16, kind="ExternalInput")
            for h in range(2)]
    vaug_d = nc.dram_tensor("vaug", [128, 32 * 130], bf16, kind="ExternalInput")
    vsaug_d = nc.dram_tensor("vsaug", [128, 4 * 130], bf16, kind="ExternalInput")
    vtail_d = nc.dram_tensor("vtail", [64, 32 * 130], bf16, kind="ExternalInput")
    # transposed output with rowsums: rows h*65..h*65+63 = O^T, row h*65+64 =
    # softmax denominators; host divides + transposes back.
    outT_d = nc.dram_tensor("outT", [130, SEQ], f32, kind="ExternalOutput")

    # Device-constant tiles (same on every core).
    import ml_dtypes
    bf = ml_dtypes.bfloat16
    n = np.arange(64)
    tri = (n[None, :] >= n[:, None]).astype(np.float32)
    m01_np = np.zeros((128, 128), np.float32)
    m01_np[:64, :64] = tri          # q-block 2p vs k-block 2p
    m01_np[:64, 64:] = 1.0          # q-block 2p+1 vs k-block 2p
    m01_np[64:, 64:] = tri          # q-block 2p+1 vs k-block 2p+1
    mB_np = np.zeros((128, 64), np.float32)
    mB_np[64:] = 1.0                # q-block 2p+8: only k-block 2p+1 valid
    m01_d = nc.inline_tensor(m01_np.astype(bf), "m01_c")
    mB_d = nc.inline_tensor(mB_np.astype(bf), "mB_c")

    with tile.TileContext(nc) as tc, ExitStack() as ctx:
        const = ctx.enter_context(tc.tile_pool(name="const", bufs=1))
        m01 = const.tile([128, 128], bf16, tag="m01")
        mB = const.tile([128, 64], bf16, tag="mB")
        nc.sync.dma_start(m01[:], m01_d[:])
        nc.sync.dma_start(mB[:], mB_d[:])

        big = ctx.enter_context(tc.tile_pool(name="big", bufs=1))
        # quarters (1024 cols) so chunk 0 starts after ~1/4 of the load;
        # issue on BOTH hwdge queues (sync + scalar) in need-order.
        kTq = [big.tile([128, 1024], bf16, tag=f"kT{i}", name=f"kT{i}")
               for i in range(4)]
        qTq = [big.tile([128, 1024], bf16, tag=f"qT{i}", name=f"qT{i}")
               for i in range(4)]
        qsh = [[big.tile([KSP, HSEQ], bf16, tag=f"qs{h}_{i}", name=f"qs{h}_{i}")
                for i in range(2)] for h in range(2)]
        ks = [big.tile([KSP, NSB * BS], bf16, tag=f"ks{h}", name=f"ks{h}")
              for h in range(2)]
        vaugh = [big.tile([128, 16 * 130], bf16, tag=f"vaug{i}", name=f"vaug{i}")
                 for i in range(2)]
        vsaug = big.tile([128, 4 * 130], bf16, tag="vsaug")
        vtailh = [big.tile([64, 16 * 130], bf16, tag=f"vtail{i}", name=f"vtail{i}")
                  for i in range(2)]
        # Two DMA queues in parallel, each loaded in need-order. The scalar
        # sequencer must finish its issues before the first exp (~17us in),
        # which 9 issues (~6us) comfortably do; its queue then transfers the
        # later-chunk tensors while sync's queue feeds the early chunks.
        for h in range(2):
            nc.scalar.dma_start(ks[h][:], ks_d[h][:])
        nc.scalar.dma_start(vsaug[:], vsaug_d[:])
        nc.scalar.dma_start(vaugh[0][:], vaug_d[:, 0:16 * 130])
        nc.scalar.dma_start(vtailh[0][:], vtail_d[:, 0:16 * 130])
        nc.scalar.dma_start(kTq[2][:], kT_d[:, 2048:3072])
        nc.scalar.dma_start(qTq[2][:], qT_d[:, 2048:3072])
        nc.scalar.dma_start(vaugh[1][:], vaug_d[:, 16 * 130:32 * 130])
        nc.scalar.dma_start(vtailh[1][:], vtail_d[:, 16 * 130:32 * 130])
        nc.sync.dma_start(kTq[0][:], kT_d[:, 0:1024])
        nc.sync.dma_start(qTq[0][:], qT_d[:, 0:1024])
        for h in range(2):
            nc.sync.dma_start(qsh[h][0][:], qs_d[h][:, 0:HSEQ])
        nc.sync.dma_start(kTq[1][:], kT_d[:, 1024:2048])
        nc.sync.dma_start(qTq[1][:], qT_d[:, 1024:2048])
        for h in range(2):
            nc.sync.dma_start(qsh[h][1][:], qs_d[h][:, HSEQ:SEQ])
        nc.sync.dma_start(kTq[3][:], kT_d[:, 3072:SEQ])
        nc.sync.dma_start(qTq[3][:], qT_d[:, 3072:SEQ])

        def kT_ap(hq, col, w):
            return kTq[col // 1024][hq, col % 1024:col % 1024 + w]

        def qT_ap(hq, col, w):
            return qTq[col // 1024][hq, col % 1024:col % 1024 + w]

        def vaug_ap(np_, p, off, w):
            i, base = (0, 0) if p < 16 else (1, 16)
            return vaugh[i][:np_, (p - base) * 130 + off:(p - base) * 130 + off + w]

        def vtail_ap(np_, p, off, w):
            i, base = (0, 0) if p < 16 else (1, 16)
            return vtailh[i][:np_, (p - base) * 130 + off:(p - base) * 130 + off + w]

        # ---- chunked attention ----
        # psD tiles span TWO PSUM banks ([128, 1024]) so one exp instruction
        # serves two 512-col score groups.
        expp = ctx.enter_context(tc.tile_pool(name="expp", bufs=10))
        psD = ctx.enter_context(tc.tile_pool(name="psD", bufs=3, space="PSUM"))
        psOT = ctx.enter_context(tc.tile_pool(name="psOT", bufs=2, space="PSUM"))
        sot = ctx.enter_context(tc.tile_pool(name="sot", bufs=2))

        for c in (range(NCH) if chunks is None else chunks):
            # phase 1: scores + exp (+ masks) for BOTH heads, so the PE can
            # stream head 1's scores while head 0's exps drain on ACT.
            pieces_h = {}
            for h in heads:
                hq = slice(h * 64, (h + 1) * 64)
                hv = h * 65
                pieces = pieces_h[h] = []  # (et ap, vl ap, ot_col, w)

                # build 512-col sub-bank groups first, then pair them into
                # two-bank [128, 1024] PSUM tiles with ONE exp each.
                # strided pieces (full-width; boundary masking rides in the
                # contraction via ks indicator rows x qs -1e9 rows):
                qschunk = qsh[h][0 if c < 4 else 1][
                    :, (c % 4) * CHUNK:(c % 4 + 1) * CHUNK]
                subbanks = []  # (used, [(p, wlo, whi, npart, kind, off)])
                for pr in range((c + 1) // 2):
                    npart = 128 if 2 * pr + 1 < c else 64
                    subbanks.append((CHUNK, [(pr, 8 * c, 8 * c + 7, npart, 3, 0)]))
                # local: one piece per k-pair window (+ tail)
                locs = []  # (p, wlo, whi, npart, kind)
                for p in range(max(0, 4 * c - 4), min(31, 4 * c + 3) + 1):
                    if p == 4 * c - 4:
                        locs.append((p, 8 * c, 8 * c, 64, 2))     # tail
                        continue
                    wlo = max(8 * c, 2 * p)
                    whi = min(8 * c + 7, 2 * p + 8, 63)
                    if wlo > whi:
                        continue
                    locs.append((p, wlo, whi, 128, 0 if p >= 4 * c else 1))
                banks = []  # [used, [(p, wlo, whi, npart, kind, off), ...]]
                for ent in sorted(locs, key=lambda e: -(e[2] - e[1] + 1)):
                    w = (ent[2] - ent[1] + 1) * 64
                    for bk in banks:
                        if bk[0] + w <= CHUNK:
                            bk[1].append(ent + (bk[0],))
                            bk[0] += w
                            break
                    else:
                        banks.append([w, [ent + (0,)]])
                subbanks.extend((u, s) for u, s in banks)

                for g in range(0, len(subbanks), 2):
                    pair = subbanks[g:g + 2]
                    ps = psD.tile([128, 2 * CHUNK], f32, tag="psD")
                    et = expp.tile([128, 2 * CHUNK], bf16, tag="exp")
                    for half, (used, subs) in enumerate(pair):
                        hb = half * CHUNK
                        for p, wlo, whi, npart, kind, off in subs:
                            w = (whi - wlo + 1) * 64
                            if kind == 3:
                                nc.tensor.matmul(
                                    ps[:npart, hb:hb + CHUNK],
                                    ks[h][:, p * 128:p * 128 + npart],
                                    qschunk, start=True, stop=True,
                                    skip_group_check=True)
                            else:
                                lhs = (kT_ap(hq, (2 * p + 1) * 64, 64)
                                       if kind == 2 else
                                       kT_ap(hq, 2 * p * 64, 128))
                                nc.tensor.matmul(
                                    ps[:npart, hb + off:hb + off + w], lhs,
                                    qT_ap(hq, wlo * 64, w),
                                    start=True, stop=True,
                                    skip_group_check=True)
                    width = (CHUNK + pair[1][0]) if len(pair) == 2 else pair[0][0]
                    nc.scalar.activation(et[:, :width], ps[:, :width], Exp,
                                         scale=SM_SCALE)
                    for half, (used, subs) in enumerate(pair):
                        hb = half * CHUNK
                        for p, wlo, whi, npart, kind, off in subs:
                            w = (whi - wlo + 1) * 64
                            if kind == 0:
                                nc.vector.tensor_mul(
                                    et[:, hb + off:hb + off + 128],
                                    et[:, hb + off:hb + off + 128], m01[:])
                            elif kind == 1:
                                nc.vector.tensor_mul(
                                    et[:, hb + off + w - 64:hb + off + w],
                                    et[:, hb + off + w - 64:hb + off + w],
                                    mB[:])
                            if kind == 3:
                                vl = vsaug[:npart, p * 130 + hv:p * 130 + hv + 65]
                            elif kind == 2:
                                vl = vtail_ap(npart, p, hv, 65)
                            else:
                                vl = vaug_ap(npart, p, hv, 65)
                            pieces.append((et[:npart, hb + off:hb + off + w], vl,
                                           (wlo - 8 * c) * 64, w))

            # phase 2: PV accumulation + output, per head. First piece must
            # cover the full 512 cols (start=True replaces a memset).
            for h in heads:
                pieces = pieces_h[h]
                ot = psOT.tile([65, CHUNK], f32, tag="psOT")
                ffull = next(i for i, pc in enumerate(pieces) if pc[3] == CHUNK)
                pieces[0], pieces[ffull] = pieces[ffull], pieces[0]
                for pi, (et, vl, col, w) in enumerate(pieces):
                    nc.tensor.matmul(ot[:, col:col + w], vl, et,
                                     start=(pi == 0), stop=(pi == len(pieces) - 1),
                                     skip_group_check=True)

                # [O^T; rowsums] to DRAM via SBUF staging; host normalizes.
                so = sot.tile([65, CHUNK], f32, tag="sot")
                nc.vector.tensor_copy(so[:], ot[:])
                nc.sync.dma_start(
                    outT_d[h * 65:(h + 1) * 65, c * CHUNK:(c + 1) * CHUNK],
                    so[:])

    return nc


def _in_maps(q, k, v):
    import ml_dtypes
    bf = ml_dtypes.bfloat16
    maps = []
    for c in range(NCORES):
        heads = [c, c + 8]
        s = (7 - c) % 8
        qT = np.ascontiguousarray(q[:, heads, :].reshape(SEQ, 128).T).astype(bf)
        kT = np.ascontiguousarray(k[:, heads, :].reshape(SEQ, 128).T).astype(bf)
        # strided contraction operands with boundary-bias augmentation:
        # ks[h] rows 64+b indicate packed block b's columns; qs[h] rows 64+b
        # carry -1e9 on chunk b+1's first s*64 columns.
        ksb = np.zeros((NSB * BS, 128), np.float32)
        vsb = np.zeros((NSB, BS, 128), np.float32)
        for b in range(7):
            j = s + 8 * b
            ksb[b * BS:(b + 1) * BS] = k[j * BS:(j + 1) * BS, heads, :].reshape(BS, 128)
            vsb[b] = v[j * BS:(j + 1) * BS, heads, :].reshape(BS, 128)
        ind = np.zeros((7, NSB * BS), np.float32)
        for b in range(7):
            ind[b, b * BS:(b + 1) * BS] = 1.0
        wrow = np.zeros((7, SEQ), np.float32)
        for b in range(7):
            wrow[b, (b + 1) * CHUNK:(b + 1) * CHUNK + s * 64] = -1e9
        qsl, ksl = [], []
        for hh in range(2):
            qs_h = np.concatenate(
                [q[:, heads[hh], :].T.astype(np.float32), wrow], axis=0)
            ks_h = np.concatenate(
                [ksb[:, hh * 64:(hh + 1) * 64].T, ind], axis=0)
            qsl.append(np.ascontiguousarray(qs_h).astype(bf))
            ksl.append(np.ascontiguousarray(ks_h).astype(bf))
        # vaug [128, 32*130]: pair a, token p -> [V_h0 | 1 | V_h1 | 1]
        vv = v[:, heads, :].reshape(32, 128, 128)   # [a, p, hd]
        vaug = np.ones((128, 32, 130), np.float32)
        vaug[:, :, 0:64] = vv.transpose(1, 0, 2)[:, :, 0:64]
        vaug[:, :, 65:129] = vv.transpose(1, 0, 2)[:, :, 64:128]
        # vsaug [128, 4*130]: pair pr: partitions 0-63 = block 2pr, 64-127 =
        # block 2pr+1
        vsp = vsb.reshape(4, 2, BS, 128).transpose(1, 2, 0, 3).reshape(128, 4, 128)
        vsaug = np.ones((128, 4, 130), np.float32)
        vsaug[:, :, 0:64] = vsp[:, :, 0:64]
        vsaug[:, :, 65:129] = vsp[:, :, 64:128]
        # vtail [64, 32*130]: odd blocks 2a+1
        vt = v[:, heads, :].reshape(32, 2, 64, 128)[:, 1]   # [a, p, hd]
        vtail = np.ones((64, 32, 130), np.float32)
        vtail[:, :, 0:64] = vt.transpose(1, 0, 2)[:, :, 0:64]
        vtail[:, :, 65:129] = vt.transpose(1, 0, 2)[:, :, 64:128]
        maps.append({"qT": qT, "kT": kT,
                     "qs0": qsl[0], "qs1": qsl[1],
                     "ks0": ksl[0], "ks1": ksl[1],
                     "vaug": vaug.reshape(128, 32 * 130).astype(bf),
                     "vsaug": vsaug.reshape(128, 4 * 130).astype(bf),
                     "vtail": vtail.reshape(64, 32 * 130).astype(bf)})
    return maps


def kernel(q, k, v, cu_seqlens_k=None, **_):
    from concourse.bass_utils import run_bass_kernel_spmd

    q = np.asarray(q, np.float32)
    k = np.asarray(k, np.float32)
    v = np.asarray(v, np.float32)
    if "nc" not in _cache:
        _cache["nc"] = _legalize_waits(_build_program())
    res = run_bass_kernel_spmd(_cache["nc"], _in_maps(q, k, v),
                               list(range(NCORES))).results
    out = np.empty((SEQ, N_HEADS, HEAD), np.float32)
    for c in range(NCORES):
        o = res[c]["outT"]                      # [130, SEQ]
        for hh, head in ((0, c), (1, c + 8)):
            num = o[hh * 65:hh * 65 + 64, :]    # [64, SEQ]
            den = o[hh * 65 + 64, :]            # [SEQ]
            out[:, head, :] = (num / den).T
    return out


# revision 28
# speedup vs baseline: 1.2153x; 1.2153x over previous
"""Block-sparse local+strided attention (LocalStridedBlockSparseAttn) on 8 trn2 cores.

Problem: q,k,v [4096, 16, 64] f32, single prefill sequence. Per-head block mask
(64x64 token blocks): j <= i and (i - j < 8  or  (j + h + 1) % 8 == 0).

Sharding: core c owns heads {c, c+8} - both have the same strided residue
s = (7 - c) % 8, so one SPMD program serves all 8 cores with per-core data.

v4 (instruction-minimal dataflow; v3 was PE-bound on per-instruction fixed
costs and startup DMA serialization):
  - local part per (chunk, head): one [128, w] matmul per k-block PAIR over
    its contiguous valid q-window (w up to 512), masked post-exp with small
    constant masks, plus one 64x64 tail piece; small pieces packed into
    shared PSUM banks so ONE activation serves several matmuls.
  - strided validity boundary folded into the CONTRACTION: ks carries 7
    indicator partitions and the strided q copy carries -1e9 rows on the
    boundary-chunk prefix columns (zero per-piece instructions).
  - output stays TRANSPOSED with the rowsums row: the [65, 512] PSUM tile
    [O^T; rowsums] is DMA'd straight to DRAM; the host does the divide and
    the final transpose. No PE transposes, no reciprocal/normalize/copy
    instructions on device.
  - big inputs split in halves with chunk-0-critical slices DMA'd first so
    compute starts before the tail of the input load.
All matmul operands bf16; exp outputs bf16 (PSUM accumulates fp32).
"""

import numpy as np

N_HEADS = 16
HEAD = 64
SEQ = 4096
BS = 64
NB = 64          # 64 token-blocks
LOCAL = 8
VERT = 8
SM_SCALE = 1.0 / 8.0
NCORES = 8
CHUNK = 512      # q tokens per chunk (8 blocks)
NCH = SEQ // CHUNK
NSB = 8          # packed strided block slots (7 real, slot 7 zero pad)
KSP = 64 + 7     # ks/qs partitions: 64 head dims + 7 boundary indicator rows
HSEQ = SEQ // 2

_cache = {}


def _legalize_waits(nc, max_waits=1):
    """This walrus build rejects instructions carrying more than one sync-wait
    condition ("Too many sync wait commands"); hoist extras into same-engine
    NoOps placed immediately before the instruction."""
    import concourse.mybir as mybir

    nid = 0
    for bb in nc.main_func.blocks:
        new = []
        for ins in bb.instructions:
            si = ins.sync_info
            if si is not None and si.on_wait and len(si.on_wait) > max_waits:
                waits = list(si.on_wait)
                while len(waits) > max_waits:
                    chunk, waits = waits[:max_waits], waits[max_waits:]
                    nid += 1
                    nop = mybir.InstNoOp(name=f"{ins.name}-wsplit{nid}")
                    nop.engine = ins.engine
                    nop.sync_info = mybir.SyncInfo(on_wait=chunk, on_update=[])
                    new.append(nop)
                ins.sync_info = mybir.SyncInfo(on_wait=waits,
                                               on_update=list(si.on_update))
            new.append(ins)
        bb.instructions[:] = new
    return nc


def _build_program(chunks=None, heads=(0, 1)):
    from contextlib import ExitStack

    import concourse.bass as bass
    import concourse.mybir as mybir
    from concourse import tile

    f32 = mybir.dt.float32
    bf16 = mybir.dt.bfloat16
    Exp = mybir.ActivationFunctionType.Exp

    nc = bass.Bass()
    qT_d = nc.dram_tensor("qT", [128, SEQ], bf16, kind="ExternalInput")
    kT_d = nc.dram_tensor("kT", [128, SEQ], bf16, kind="ExternalInput")
    ksT_d = nc.dram_tensor("ksT", [128, NSB * BS], bf16, kind="ExternalInput")
    vaug_d = nc.dram_tensor("vaug", [128, 32 * 130], bf16, kind="ExternalInput")
    vsaug_d = nc.dram_tensor("vsaug", [128, 4 * 130], bf16, kind="ExternalInput")
    # per-core strided boundary masks (all-ones prefix zeroed up to s*64):
    # mbH for a boundary block in the pair's high half, mbL for a lone one.
    mbH_d = nc.dram_tensor("mbH", [128, CHUNK], bf16, kind="ExternalInput")
    mbL_d = nc.dram_tensor("mbL", [64, CHUNK], bf16, kind="ExternalInput")
    # transposed output with rowsums: rows h*65..h*65+63 = O^T, row h*65+64 =
    # softmax denominators; host divides + transposes back.
    outT_d = nc.dram_tensor("outT", [130, SEQ], f32, kind="ExternalOutput")

    # Device-constant tiles (same on every core).
    import ml_dtypes
    bf = ml_dtypes.bfloat16
    n = np.arange(64)
    tri = (n[None, :] >= n[:, None]).astype(np.float32)
    m01_np = np.zeros((128, 128), np.float32)
    m01_np[:64, :64] = tri          # q-block 2p vs k-block 2p
    m01_np[:64, 64:] = 1.0          # q-block 2p+1 vs k-block 2p
    m01_np[64:, 64:] = tri          # q-block 2p+1 vs k-block 2p+1
    mB_np = np.zeros((128, 64), np.float32)
    mB_np[64:] = 1.0                # q-block 2p+8: only k-block 2p+1 valid
    m01_d = nc.inline_tensor(m01_np.astype(bf), "m01_c")
    mB_d = nc.inline_tensor(mB_np.astype(bf), "mB_c")

    with tile.TileContext(nc) as tc, ExitStack() as ctx:
        const = ctx.enter_context(tc.tile_pool(name="const", bufs=1))
        m01 = const.tile([128, 128], bf16, tag="m01")
        mB = const.tile([128, 64], bf16, tag="mB")
        nc.sync.dma_start(m01[:], m01_d[:])
        nc.sync.dma_start(mB[:], mB_d[:])

        big = ctx.enter_context(tc.tile_pool(name="big", bufs=1))
        # quarters (1024 cols) so chunk 0 starts after ~1/4 of the load;
        # issue on BOTH hwdge queues (sync + scalar) in need-order.
        kTq = [big.tile([128, 1024], bf16, tag=f"kT{i}", name=f"kT{i}")
               for i in range(4)]
        qTq = [big.tile([128, 1024], bf16, tag=f"qT{i}", name=f"qT{i}")
               for i in range(4)]
        ksT = big.tile([128, NSB * BS], bf16, tag="ksT")
        vaugh = [big.tile([128, 16 * 130], bf16, tag=f"vaug{i}", name=f"vaug{i}")
                 for i in range(2)]
        vsaug = big.tile([128, 4 * 130], bf16, tag="vsaug")
        mbH = big.tile([128, CHUNK], bf16, tag="mbH")
        mbL = big.tile([64, CHUNK], bf16, tag="mbL")
        # scalar queue: small early tensors + chunk-0/1 V layouts (the scalar
        # sequencer must drain its DMA issues before it can run the first exp)
        nc.scalar.dma_start(ksT[:], ksT_d[:])
        nc.scalar.dma_start(vsaug[:], vsaug_d[:])
        nc.scalar.dma_start(mbH[:], mbH_d[:])
        nc.scalar.dma_start(mbL[:], mbL_d[:])
        nc.scalar.dma_start(vaugh[0][:], vaug_d[:, 0:16 * 130])
        # sync queue: everything else in need-order
        nc.sync.dma_start(kTq[0][:], kT_d[:, 0:1024])
        nc.sync.dma_start(qTq[0][:], qT_d[:, 0:1024])
        nc.sync.dma_start(kTq[1][:], kT_d[:, 1024:2048])
        nc.sync.dma_start(qTq[1][:], qT_d[:, 1024:2048])
        nc.sync.dma_start(kTq[2][:], kT_d[:, 2048:3072])
        nc.sync.dma_start(qTq[2][:], qT_d[:, 2048:3072])
        nc.sync.dma_start(vaugh[1][:], vaug_d[:, 16 * 130:32 * 130])
        nc.sync.dma_start(kTq[3][:], kT_d[:, 3072:SEQ])
        nc.sync.dma_start(qTq[3][:], qT_d[:, 3072:SEQ])

        def kT_ap(hq, col, w):
            return kTq[col // 1024][hq, col % 1024:col % 1024 + w]

        def qT_ap(hq, col, w):
            return qTq[col // 1024][hq, col % 1024:col % 1024 + w]

        def vaug_ap(np_, p, off, w):
            i, base = (0, 0) if p < 16 else (1, 16)
            return vaugh[i][:np_, (p - base) * 130 + off:(p - base) * 130 + off + w]

        def vaug_ap2(p, off):
            # high half of pair p = tokens of odd block 2p+1 (tail PV lhsT)
            i, base = (0, 0) if p < 16 else (1, 16)
            return vaugh[i][64:128, (p - base) * 130 + off:(p - base) * 130 + off + 65]

        # ---- chunked attention ----
        # psD tiles span TWO PSUM banks ([128, 1024]) so one exp instruction
        # serves two 512-col score groups.
        expp = ctx.enter_context(tc.tile_pool(name="expp", bufs=10))
        psD = ctx.enter_context(tc.tile_pool(name="psD", bufs=3, space="PSUM"))
        psOT = ctx.enter_context(tc.tile_pool(name="psOT", bufs=2, space="PSUM"))
        sot = ctx.enter_context(tc.tile_pool(name="sot", bufs=2))

        for c in (range(NCH) if chunks is None else chunks):
            # phase 1: scores + exp (+ masks) for BOTH heads, so the PE can
            # stream head 1's scores while head 0's exps drain on ACT.
            pieces_h = {}
            for h in heads:
                hq = slice(h * 64, (h + 1) * 64)
                hv = h * 65
                pieces = pieces_h[h] = []  # (et ap, vl ap, ot_col, w)

                # build 512-col sub-bank groups first, then pair them into
                # two-bank [128, 1024] PSUM tiles with ONE exp each.
                # strided pieces (full-width; the boundary block's invalid
                # prefix columns are zeroed post-exp with a per-core mask):
                subbanks = []  # (used, [(p, wlo, whi, npart, kind, off)])
                for pr in range((c + 1) // 2):
                    npart = 128 if 2 * pr + 1 < c else 64
                    subbanks.append((CHUNK, [(pr, 8 * c, 8 * c + 7, npart, 3, 0)]))
                # local: one piece per k-pair window (+ tail)
                locs = []  # (p, wlo, whi, npart, kind)
                for p in range(max(0, 4 * c - 4), min(31, 4 * c + 3) + 1):
                    if p == 4 * c - 4:
                        locs.append((p, 8 * c, 8 * c, 64, 2))     # tail
                        continue
                    wlo = max(8 * c, 2 * p)
                    whi = min(8 * c + 7, 2 * p + 8, 63)
                    if wlo > whi:
                        continue
                    locs.append((p, wlo, whi, 128, 0 if p >= 4 * c else 1))
                banks = []  # [used, [(p, wlo, whi, npart, kind, off), ...]]
                for ent in sorted(locs, key=lambda e: -(e[2] - e[1] + 1)):
                    w = (ent[2] - ent[1] + 1) * 64
                    for bk in banks:
                        if bk[0] + w <= CHUNK:
                            bk[1].append(ent + (bk[0],))
                            bk[0] += w
                            break
                    else:
                        banks.append([w, [ent + (0,)]])
                subbanks.extend((u, s) for u, s in banks)

                for g in range(0, len(subbanks), 2):
                    pair = subbanks[g:g + 2]
                    ps = psD.tile([128, 2 * CHUNK], f32, tag="psD")
                    et = expp.tile([128, 2 * CHUNK], bf16, tag="exp")
                    for half, (used, subs) in enumerate(pair):
                        hb = half * CHUNK
                        for p, wlo, whi, npart, kind, off in subs:
                            w = (whi - wlo + 1) * 64
                            if kind == 3:
                                nc.tensor.matmul(
                                    ps[:npart, hb:hb + CHUNK],
                                    ksT[hq, p * 128:p * 128 + npart],
                                    qT_ap(hq, 8 * c * 64, CHUNK),
                                    start=True, stop=True,
                                    skip_group_check=True)
                            elif kind == 2:
                                # tail targets partitions 64-127 so its PV
                                # can use vaug's high half directly
                                nc.tensor.matmul(
                                    ps[64:128, hb + off:hb + off + w],
                                    kT_ap(hq, (2 * p + 1) * 64, 64),
                                    qT_ap(hq, wlo * 64, w),
                                    start=True, stop=True,
                                    skip_group_check=True)
                            else:
                                nc.tensor.matmul(
                                    ps[:npart, hb + off:hb + off + w],
                                    kT_ap(hq, 2 * p * 64, 128),
                                    qT_ap(hq, wlo * 64, w),
                                    start=True, stop=True,
                                    skip_group_check=True)
                    width = (CHUNK + pair[1][0]) if len(pair) == 2 else pair[0][0]
                    nc.scalar.activation(et[:, :width], ps[:, :width], Exp,
                                         scale=SM_SCALE)
                    for half, (used, subs) in enumerate(pair):
                        hb = half * CHUNK
                        for p, wlo, whi, npart, kind, off in subs:
                            w = (whi - wlo + 1) * 64
                            if kind == 0:
                                nc.vector.tensor_mul(
                                    et[:, hb + off:hb + off + 128],
                                    et[:, hb + off:hb + off + 128], m01[:])
                            elif kind == 1:
                                nc.vector.tensor_mul(
                                    et[:, hb + off + w - 64:hb + off + w],
                                    et[:, hb + off + w - 64:hb + off + w],
                                    mB[:])
                            elif kind == 3 and p == (c - 1) // 2:
                                # strided boundary block: zero the invalid
                                # prefix columns (per-core mask data)
                                if npart == 64:
                                    nc.vector.tensor_mul(
                                        et[:64, hb:hb + CHUNK],
                                        et[:64, hb:hb + CHUNK], mbL[:])
                                else:
                                    nc.vector.tensor_mul(
                                        et[:, hb:hb + CHUNK],
                                        et[:, hb:hb + CHUNK], mbH[:])
                            if kind == 3:
                                vl = vsaug[:npart, p * 130 + hv:p * 130 + hv + 65]
                                ea = et[:npart, hb:hb + CHUNK]
                            elif kind == 2:
                                vl = vaug_ap2(p, hv)
                                ea = et[64:128, hb + off:hb + off + w]
                            else:
                                vl = vaug_ap(npart, p, hv, 65)
                                ea = et[:npart, hb + off:hb + off + w]
                            pieces.append((ea, vl, (wlo - 8 * c) * 64, w))

            # phase 2: PV accumulation + output, per head. First piece must
            # cover the full 512 cols (start=True replaces a memset).
            for h in heads:
                pieces = pieces_h[h]
                ot = psOT.tile([65, CHUNK], f32, tag="psOT")
                ffull = next(i for i, pc in enumerate(pieces) if pc[3] == CHUNK)
                pieces[0], pieces[ffull] = pieces[ffull], pieces[0]
                for pi, (et, vl, col, w) in enumerate(pieces):
                    nc.tensor.matmul(ot[:, col:col + w], vl, et,
                                     start=(pi == 0), stop=(pi == len(pieces) - 1),
                                     skip_group_check=True)

                # [O^T; rowsums] to DRAM via SBUF staging; host normalizes.
                so = sot.tile([65, CHUNK], f32, tag="sot")
                nc.vector.tensor_copy(so[:], ot[:])
                nc.sync.dma_start(
                    outT_d[h * 65:(h + 1) * 65, c * CHUNK:(c + 1) * CHUNK],
                    so[:])

    return nc


def _in_maps(q, k, v):
    import ml_dtypes
    bf = ml_dtypes.bfloat16
    maps = []
    for c in range(NCORES):
        heads = [c, c + 8]
        s = (7 - c) % 8
        qT = np.ascontiguousarray(q[:, heads, :].reshape(SEQ, 128).T).astype(bf)
        kT = np.ascontiguousarray(k[:, heads, :].reshape(SEQ, 128).T).astype(bf)
        # packed strided k blocks (7 real + zero pad), transposed, and the
        # boundary masks (zero the first s*64 columns of the boundary chunk)
        ksb = np.zeros((NSB * BS, 128), np.float32)
        vsb = np.zeros((NSB, BS, 128), np.float32)
        for b in range(7):
            j = s + 8 * b
            ksb[b * BS:(b + 1) * BS] = k[j * BS:(j + 1) * BS, heads, :].reshape(BS, 128)
            vsb[b] = v[j * BS:(j + 1) * BS, heads, :].reshape(BS, 128)
        ksT = np.ascontiguousarray(ksb.T).astype(bf)
        mbH = np.ones((128, CHUNK), np.float32)
        mbH[64:, :s * 64] = 0.0
        mbL = np.ones((64, CHUNK), np.float32)
        mbL[:, :s * 64] = 0.0
        # vaug [128, 32*130]: pair a, token p -> [V_h0 | 1 | V_h1 | 1]
        vv = v[:, heads, :].reshape(32, 128, 128)   # [a, p, hd]
        vaug = np.ones((128, 32, 130), np.float32)
        vaug[:, :, 0:64] = vv.transpose(1, 0, 2)[:, :, 0:64]
        vaug[:, :, 65:129] = vv.transpose(1, 0, 2)[:, :, 64:128]
        # vsaug [128, 4*130]: pair pr: partitions 0-63 = block 2pr, 64-127 =
        # block 2pr+1
        vsp = vsb.reshape(4, 2, BS, 128).transpose(1, 2, 0, 3).reshape(128, 4, 128)
        vsaug = np.ones((128, 4, 130), np.float32)
        vsaug[:, :, 0:64] = vsp[:, :, 0:64]
        vsaug[:, :, 65:129] = vsp[:, :, 64:128]
        maps.append({"qT": qT, "kT": kT, "ksT": ksT,
                     "vaug": vaug.reshape(128, 32 * 130).astype(bf),
                     "vsaug": vsaug.reshape(128, 4 * 130).astype(bf),
                     "mbH": mbH.astype(bf), "mbL": mbL.astype(bf)})
    return maps


def kernel(q, k, v, cu_seqlens_k=None, **_):
    from concourse.bass_utils import run_bass_kernel_spmd

    q = np.asarray(q, np.float32)
    k = np.asarray(k, np.float32)
    v = np.asarray(v, np.float32)
    if "nc" not in _cache:
        _cache["nc"] = _legalize_waits(_build_program())
    res = run_bass_kernel_spmd(_cache["nc"], _in_maps(q, k, v),
                               list(range(NCORES))).results
    out = np.empty((SEQ, N_HEADS, HEAD), np.float32)
    for c in range(NCORES):
        o = res[c]["outT"]                      # [130, SEQ]
        for hh, head in ((0, c), (1, c + 8)):
            num = o[hh * 65:hh * 65 + 64, :]    # [64, SEQ]
            den = o[hh * 65 + 64, :]            # [SEQ]
            out[:, head, :] = (num / den).T
    return out
